# revision 1
# baseline (speedup 1.0000x reference)
"""Trainium2 Bass kernel for dense transformer block nn_Block_68221260529679.

Layout: B=2, T=2048, D=2048, N=8 q-heads, K=1 kv-head, H=256, F=16384.

Sharding (8 NeuronCores): DP over batch (2 groups of 4) x TP within group (4).
Core c = 4*b + r: batch b, q-heads {HEADS*r .. HEADS*(r+1)}, MLP hidden rows
[r*FS, (r+1)*FS).  Within each 4-core group:
  - every core computes the full rmsnorm(x) and the shared k/v projection
    (K=1 kv-head) redundantly,
  - attention + o-proj partial summed over the core's heads -> ReduceScatter
    (each core keeps T-slice r),
  - x2 = x + attn on the slice, rmsnorm, transpose -> AllGather of h2^T,
  - gate/up/gelu/down on the F-shard over all T -> ReduceScatter,
  - out slice = x2 + down.  Host assembles the 8 [T/4, D] slices.

All matmuls in bf16 with fp32 PSUM accumulation; norms/softmax/residuals fp32.
The rmsnorm scales (1+scale) and the q scaling H^-0.5 are folded into the
weights host-side; rope sin/cos tables and the additive mask bias are
precomputed host-side per batch.  Weights are pre-laid-out host-side so every
DMA moves contiguous >=8KB lines per partition.
"""

from contextlib import ExitStack

import numpy as np
import ml_dtypes

import concourse.bass as bass
import concourse.mybir as mybir
import concourse.tile as tile
from concourse import bacc
from concourse.masks import make_identity

F32 = mybir.dt.float32
BF16 = mybir.dt.bfloat16
AF = mybir.ActivationFunctionType
ALU = mybir.AluOpType
BIG_NEG = -2.3819763e38
GROUPS = [[0, 1, 2, 3], [4, 5, 6, 7]]

FULL_CFG = dict(T=2048, D=2048, H=256, HEADS=2, FS=4096)


def build(cfg):
    T, D, H, HEADS, FS = cfg["T"], cfg["D"], cfg["H"], cfg["HEADS"], cfg["FS"]
    REPS = cfg.get("reps", 1)
    assert H == 256
    TP = 4
    TT, DT, FB = T // 128, D // 128, FS // 128
    TCH = T // TP              # attention/MLP T-chunk == rank slice
    NCH, CHT, DCH = T // TCH, TCH // 128, D // TCH
    SLICE = TCH
    ST = SLICE // 128
    BS = TCH // TP             # per-rank row block within a T-chunk
    QB = HEADS * H // 128      # q col blocks (2 per head)

    nc = bacc.Bacc("TRN2", target_bir_lowering=False, debug=False, num_devices=8)
    x_ext = nc.dram_tensor("x", [T, D], F32, kind="ExternalInput").ap()
    xs_ext = nc.dram_tensor("x_slice", [SLICE, D], F32, kind="ExternalInput").ap()
    wq_ext = nc.dram_tensor("wq", [128, DT, HEADS * H], BF16,
                            kind="ExternalInput").ap()
    wkv_ext = nc.dram_tensor("wkv", [128, DT, 2 * H], BF16,
                             kind="ExternalInput").ap()
    wo_ext = nc.dram_tensor("wo", [128, QB, D], BF16, kind="ExternalInput").ap()
    wg_ext = nc.dram_tensor("wg", [FB, 128, DT, 256], BF16,
                            kind="ExternalInput").ap()
    wl_ext = nc.dram_tensor("wl", [DCH, 128, FB, TCH], BF16,
                            kind="ExternalInput").ap()
    sin_ext = nc.dram_tensor("sin", [H // 2, T], F32, kind="ExternalInput").ap()
    cos_ext = nc.dram_tensor("cos", [H // 2, T], F32, kind="ExternalInput").ap()
    out_ext = nc.dram_tensor("out", [SLICE, D], F32, kind="ExternalOutput").ap()

    with tile.TileContext(nc) as tc, ExitStack() as top:
        cons = top.enter_context(tc.tile_pool(name="cons", bufs=1))
        dram = top.enter_context(tc.tile_pool(name="dram", bufs=1, space="DRAM"))

        ident = cons.tile([128, 128], BF16)
        make_identity(nc, ident)
        eps = cons.tile([128, 1], F32)
        nc.vector.memset(eps, 1e-6)
        sin_sb = cons.tile([128, T], F32)
        nc.sync.dma_start(out=sin_sb, in_=sin_ext)
        cos_sb = cons.tile([128, T], F32)
        nc.sync.dma_start(out=cos_sb, in_=cos_ext)

        # DRAM intermediates, one set per T-chunk (chunked collectives)
        attn_d = [dram.tile([TCH, D], BF16, tag=f"attn_d{i}", name=f"attn_d{i}") for i in range(NCH)]
        attn_r = [dram.tile([BS, D], BF16, tag=f"attn_r{i}", name=f"attn_r{i}") for i in range(NCH)]
        h2o = [dram.tile([D, BS], BF16, tag=f"h2o{i}", name=f"h2o{i}") for i in range(NCH)]
        h2g = [dram.tile([TP * D, BS], BF16, tag=f"h2g{i}", name=f"h2g{i}") for i in range(NCH)]
        down_d = [dram.tile([TCH, D], BF16, tag=f"down_d{i}", name=f"down_d{i}") for i in range(NCH)]
        down_r = [dram.tile([BS, D], BF16, tag=f"down_r{i}", name=f"down_r{i}") for i in range(NCH)]
        x2_d = [dram.tile([BS, D], F32, tag=f"x2_d{i}", name=f"x2_d{i}") for i in range(NCH)]

        for _rep in range(REPS):
            with ExitStack() as attn_sc:
                acts = attn_sc.enter_context(tc.tile_pool(name="acts", bufs=1))
                qT = acts.tile([128, 2 * HEADS, T], BF16, tag="qT")
                kT = acts.tile([128, 2, T], BF16, tag="kT")
                v_sb = acts.tile([128, TT, H], BF16, tag="v")

                # ---------------- P1: rmsnorm(x) + transpose -> hT --------
                with (
                    tc.tile_pool(name="p12", bufs=2) as p12,
                    tc.tile_pool(name="ps12", bufs=2, space="PSUM") as ps12,
                ):
                    hT = p12.tile([128, DT, T], BF16, tag="hT", bufs=1)
                    for t in range(TT):
                        tsl = slice(t * 128, (t + 1) * 128)
                        xt = p12.tile([128, D], F32, tag="xt")
                        nc.sync.dma_start(out=xt, in_=x_ext[tsl])
                        h = p12.tile([128, D], BF16, tag="h")
                        ssq = p12.tile([128, 1], F32, tag="ssq")
                        nc.scalar.activation(out=h, in_=xt, func=AF.Square,
                                             accum_out=ssq)
                        rstd = p12.tile([128, 1], F32, tag="rstd")
                        nc.scalar.activation(out=rstd, in_=ssq, func=AF.Sqrt,
                                             bias=eps, scale=1.0 / D)
                        nc.vector.reciprocal(out=rstd, in_=rstd)
                        nc.vector.tensor_scalar_mul(h, xt, rstd)
                        for kd in range(DT):
                            pt = ps12.tile([128, 128], BF16, tag="tp")
                            nc.tensor.transpose(
                                pt, h[:, kd * 128:(kd + 1) * 128], ident)
                            nc.vector.tensor_copy(hT[:, kd, tsl], pt)

                    # ------------ P2: q/k/v projections + rope ------------
                    wqs = p12.tile([128, DT, HEADS * H], BF16, tag="wqs",
                                   bufs=1)
                    nc.sync.dma_start(out=wqs, in_=wq_ext)
                    wkvs = p12.tile([128, DT, 2 * H], BF16, tag="wkvs", bufs=1)
                    nc.sync.dma_start(out=wkvs, in_=wkv_ext)

                    def rope_pair(dst, blk1, blk2, x1p, x2p, csl):
                        cs, sn = cos_sb[:, csl], sin_sb[:, csl]
                        t1 = p12.tile([128, TCH], F32, tag="rp1")
                        t2 = p12.tile([128, TCH], F32, tag="rp2")
                        nc.vector.tensor_tensor(t1, x1p, cs, op=ALU.mult)
                        nc.vector.tensor_tensor(t2, x2p, sn, op=ALU.mult)
                        nc.vector.tensor_tensor(dst[:, blk1, csl], t1, t2,
                                                op=ALU.subtract)
                        nc.vector.tensor_tensor(t1, x2p, cs, op=ALU.mult)
                        nc.vector.tensor_tensor(t2, x1p, sn, op=ALU.mult)
                        nc.vector.tensor_tensor(dst[:, blk2, csl], t1, t2,
                                                op=ALU.add)

                    for ch in range(NCH):
                        csl = slice(ch * TCH, (ch + 1) * TCH)
                        for hd in range(HEADS):   # q heads
                            p1 = ps12.tile([128, TCH], F32, tag="qk1")
                            p2 = ps12.tile([128, TCH], F32, tag="qk2")
                            for kd in range(DT):
                                c0 = (2 * hd) * 128
                                nc.tensor.matmul(p1, wqs[:, kd, c0:c0 + 128],
                                                 hT[:, kd, csl],
                                                 start=kd == 0,
                                                 stop=kd == DT - 1)
                            for kd in range(DT):
                                c1 = (2 * hd + 1) * 128
                                nc.tensor.matmul(p2, wqs[:, kd, c1:c1 + 128],
                                                 hT[:, kd, csl],
                                                 start=kd == 0,
                                                 stop=kd == DT - 1)
                            rope_pair(qT, 2 * hd, 2 * hd + 1, p1, p2, csl)
                        # k
                        p1 = ps12.tile([128, TCH], F32, tag="qk1")
                        p2 = ps12.tile([128, TCH], F32, tag="qk2")
                        for kd in range(DT):
                            nc.tensor.matmul(p1, wkvs[:, kd, 0:128],
                                             hT[:, kd, csl],
                                             start=kd == 0, stop=kd == DT - 1)
                        for kd in range(DT):
                            nc.tensor.matmul(p2, wkvs[:, kd, 128:256],
                                             hT[:, kd, csl],
                                             start=kd == 0, stop=kd == DT - 1)
                        rope_pair(kT, 0, 1, p1, p2, csl)
                        # v (natural layout [S, H])
                        for st in range(ch * CHT, (ch + 1) * CHT):
                            pv = ps12.tile([128, H], F32, tag="vv")
                            for kd in range(DT):
                                nc.tensor.matmul(
                                    pv, hT[:, kd, st * 128:(st + 1) * 128],
                                    wkvs[:, kd, H:2 * H],
                                    start=kd == 0, stop=kd == DT - 1)
                            nc.vector.tensor_copy(v_sb[:, st], pv)

                # ---------------- P3: attention + o-proj ----------------
                with (
                    tc.tile_pool(name="p3", bufs=2) as p3,
                    tc.tile_pool(name="ps3", bufs=2, space="PSUM") as ps3,
                ):
                    wos = p3.tile([128, QB, D], BF16, tag="wos", bufs=1)
                    nc.sync.dma_start(out=wos, in_=wo_ext)

                    def p4_block(ch):
                        xt = p3.tile([BS, D], F32, tag="xs")
                        nc.sync.dma_start(out=xt,
                                          in_=xs_ext[ch * BS:(ch + 1) * BS])
                        ar = p3.tile([BS, D], BF16, tag="ar")
                        nc.sync.dma_start(out=ar, in_=attn_r[ch])
                        x2t = p3.tile([BS, D], F32, tag="x2t")
                        nc.vector.tensor_tensor(x2t, xt, ar, op=ALU.add)
                        nc.sync.dma_start(out=x2_d[ch], in_=x2t)
                        h2 = p3.tile([BS, D], BF16, tag="h2")
                        ssq = p3.tile([BS, 1], F32, tag="ssq2")
                        nc.scalar.activation(out=h2, in_=x2t, func=AF.Square,
                                             accum_out=ssq)
                        rstd = p3.tile([BS, 1], F32, tag="rstd2")
                        nc.scalar.activation(out=rstd, in_=ssq, func=AF.Sqrt,
                                             bias=eps[:BS], scale=1.0 / D)
                        nc.vector.reciprocal(out=rstd, in_=rstd)
                        nc.vector.tensor_scalar_mul(h2, x2t, rstd)
                        h2tb = p3.tile([128, DT, BS], BF16, tag="h2tb")
                        for kd in range(DT):
                            pt = ps3.tile([128, BS], BF16, tag="tp")
                            nc.tensor.transpose(
                                pt, h2[:, kd * 128:(kd + 1) * 128],
                                ident[:BS, :BS])
                            nc.vector.tensor_copy(h2tb[:, kd], pt)
                        nc.sync.dma_start(
                            out=h2o[ch].rearrange("(a p) s -> p a s", p=128),
                            in_=h2tb)
                        nc.gpsimd.collective_compute(
                            "AllGather", ALU.bypass, replica_groups=GROUPS,
                            ins=[h2o[ch].opt()], outs=[h2g[ch].opt()])

                    for ch in range(NCH):
                        encT = p3.tile([128, HEADS, 2, TCH], BF16, tag="encT")
                        for hd in range(HEADS):
                            nsc = ch + 1            # causal S-chunks
                            nS = nsc * CHT          # causal S-tiles
                            pT = p3.tile([128, nS, TCH], BF16, tag="pT",
                                         bufs=1)
                            for tt in range(CHT):
                                gt = ch * CHT + tt
                                gsl = slice(gt * 128, (gt + 1) * 128)
                                seff = nsc * TCH
                                mb = p3.tile([128, T], BF16, tag="mb")
                                nc.gpsimd.memset(mb[:, :seff], 0.0)
                                nc.gpsimd.affine_select(
                                    out=mb[:, :seff], in_=mb[:, :seff],
                                    compare_op=ALU.is_ge, fill=BIG_NEG,
                                    base=gt * 128, channel_multiplier=1,
                                    pattern=[[-1, seff]])
                                lg = p3.tile([128, T], F32, tag="lg")
                                for sc in range(nsc):
                                    ssl = slice(sc * TCH, (sc + 1) * TCH)
                                    pl = ps3.tile([128, TCH], F32, tag="lgp")
                                    nc.tensor.matmul(pl, qT[:, 2 * hd, gsl],
                                                     kT[:, 0, ssl],
                                                     start=True, stop=False)
                                    nc.tensor.matmul(pl,
                                                     qT[:, 2 * hd + 1, gsl],
                                                     kT[:, 1, ssl],
                                                     start=False, stop=True)
                                    nc.vector.tensor_tensor(
                                        lg[:, ssl], pl, mb[:, ssl], op=ALU.add)
                                nmax = p3.tile([128, 1], F32, tag="nmax")
                                nc.vector.tensor_reduce(
                                    nmax, lg[:, :seff],
                                    axis=mybir.AxisListType.X,
                                    op=ALU.max, negate=True)
                                sume = p3.tile([128, 1], F32, tag="sume")
                                nc.scalar.activation(
                                    out=lg[:, :seff], in_=lg[:, :seff],
                                    func=AF.Exp, bias=nmax, accum_out=sume)
                                rsum = p3.tile([128, 1], F32, tag="rsum")
                                nc.vector.reciprocal(rsum, sume)
                                pb = p3.tile([128, T], BF16, tag="pbf")
                                nc.vector.tensor_scalar_mul(
                                    pb[:, :seff], lg[:, :seff], rsum)
                                for s in range(nS):
                                    ptp = ps3.tile([128, 128], BF16, tag="tp")
                                    nc.tensor.transpose(
                                        ptp, pb[:, s * 128:(s + 1) * 128],
                                        ident)
                                    nc.vector.tensor_copy(
                                        pT[:, s, tt * 128:(tt + 1) * 128], ptp)
                            for m in range(2):
                                pe_ = ps3.tile([128, TCH], F32, tag="enc")
                                for s in range(nS):
                                    nc.tensor.matmul(
                                        pe_,
                                        v_sb[:, s, m * 128:(m + 1) * 128],
                                        pT[:, s, :],
                                        start=s == 0, stop=s == nS - 1)
                                nc.vector.tensor_copy(encT[:, hd, m], pe_)
                        # o-proj for this chunk, summed over the core's heads
                        for tt in range(CHT):
                            gt = ch * CHT + tt
                            ao = p3.tile([128, D], BF16, tag="ao")
                            for dch in range(DCH):
                                dsl = slice(dch * TCH, (dch + 1) * TCH)
                                po = ps3.tile([128, TCH], F32, tag="oproj")
                                kk = 0
                                for hd in range(HEADS):
                                    for m in range(2):
                                        nc.tensor.matmul(
                                            po,
                                            encT[:, hd, m,
                                                 tt * 128:(tt + 1) * 128],
                                            wos[:, 2 * hd + m, dsl],
                                            start=kk == 0,
                                            stop=kk == 2 * HEADS - 1)
                                        kk += 1
                                nc.vector.tensor_copy(ao[:, dsl], po)
                            nc.sync.dma_start(
                                out=attn_d[ch][tt * 128:(tt + 1) * 128],
                                in_=ao)
                        # issue the chunk RS now; the residual/norm/AG for
                        # this chunk is emitted during the NEXT chunk's
                        # attention so the in-order engines don't stall on
                        # the collective.
                        nc.gpsimd.collective_compute(
                            "ReduceScatter", ALU.add, replica_groups=GROUPS,
                            ins=[attn_d[ch].opt()], outs=[attn_r[ch].opt()])
                        if ch > 0:
                            p4_block(ch - 1)
                    p4_block(NCH - 1)

            # ------------ P5: MLP on F-shard over all T ------------
            with (
                tc.tile_pool(name="p5", bufs=2) as p5,
                tc.tile_pool(name="ps5", bufs=2, space="PSUM") as ps5,
            ):
                def final_block(r):
                    dr = p5.tile([BS, D], BF16, tag="dr", bufs=1)
                    nc.sync.dma_start(out=dr, in_=down_r[r])
                    x2f = p5.tile([BS, D], F32, tag="x2f", bufs=1)
                    nc.sync.dma_start(out=x2f, in_=x2_d[r])
                    ot = p5.tile([BS, D], F32, tag="ot", bufs=1)
                    nc.vector.tensor_tensor(ot, x2f, dr, op=ALU.add)
                    nc.sync.dma_start(out=out_ext[r * BS:(r + 1) * BS],
                                      in_=ot)

                for r in range(NCH):
                    h2c = p5.tile([128, DT, TCH], BF16, tag="h2c")
                    for j in range(TP):
                        nc.sync.dma_start(
                            out=h2c[:, :, j * BS:(j + 1) * BS],
                            in_=h2g[r][j * D:(j + 1) * D].rearrange(
                                "(a p) s -> p a s", p=128))
                    ffT = p5.tile([128, FB, TCH], BF16, tag="ffT", bufs=1)
                    for f in range(FB):
                        wgf = p5.tile([128, DT, 256], BF16, tag="wgf", bufs=3)
                        nc.sync.dma_start(out=wgf, in_=wg_ext[f])
                        gps = ps5.tile([128, TCH], F32, tag="gps")
                        ups = ps5.tile([128, TCH], F32, tag="ups")
                        for kd in range(DT):
                            nc.tensor.matmul(gps, wgf[:, kd, 0:128],
                                             h2c[:, kd],
                                             start=kd == 0, stop=kd == DT - 1)
                        for kd in range(DT):
                            nc.tensor.matmul(ups, wgf[:, kd, 128:256],
                                             h2c[:, kd],
                                             start=kd == 0, stop=kd == DT - 1)
                        ga = p5.tile([128, TCH], F32, tag="ga")
                        nc.scalar.activation(out=ga, in_=gps,
                                             func=AF.Gelu_apprx_tanh)
                        nc.vector.tensor_tensor(ffT[:, f], ga, ups,
                                                op=ALU.mult)
                    for dch in range(DCH):
                        dsl = slice(dch * TCH, (dch + 1) * TCH)
                        wlc = p5.tile([128, FB, TCH], BF16, tag="wlc", bufs=2)
                        nc.sync.dma_start(out=wlc, in_=wl_ext[dch])
                        for tt in range(CHT):
                            dps = ps5.tile([128, TCH], F32, tag=f"dps{tt}",
                                           bufs=1)
                            for f in range(FB):
                                nc.tensor.matmul(
                                    dps, ffT[:, f, tt * 128:(tt + 1) * 128],
                                    wlc[:, f],
                                    start=f == 0, stop=f == FB - 1)
                            od = p5.tile([128, TCH], BF16, tag="od", bufs=3)
                            nc.scalar.copy(out=od, in_=dps)
                            nc.sync.dma_start(
                                out=down_d[r][tt * 128:(tt + 1) * 128, dsl],
                                in_=od)
                    nc.gpsimd.collective_compute(
                        "ReduceScatter", ALU.add, replica_groups=GROUPS,
                        ins=[down_d[r].opt()], outs=[down_r[r].opt()])
                    if r > 0:
                        final_block(r - 1)
                final_block(NCH - 1)
    nc.compile()
    return nc


# ---------------------------------------------------------------------------
# host side
# ---------------------------------------------------------------------------

def _pa(w, inner=128):
    """[A*128, N] -> [128, A, N] partition-major layout."""
    a = w.shape[0] // inner
    return np.ascontiguousarray(
        w.reshape(a, inner, w.shape[1]).transpose(1, 0, 2))


def make_in_maps(cfg, x, positions, attn_mask, scale_attn, w_q, w_kv, w_o,
                 scale_ffn, w_gating, w_linear):
    T, D, H, HEADS, FS = cfg["T"], cfg["D"], cfg["H"], cfg["HEADS"], cfg["FS"]
    SLICE = T // 4
    TCH = SLICE
    NCH = T // TCH
    BS = TCH // 4
    DT, FB, DCH = D // 128, FS // 128, D // TCH
    bf = ml_dtypes.bfloat16
    s1a = (1.0 + np.asarray(scale_attn, np.float32))[:, None]
    s1f = (1.0 + np.asarray(scale_ffn, np.float32))[:, None]
    k_w = (np.asarray(w_kv[0, 0], np.float32) * s1a)
    v_w = (np.asarray(w_kv[1, 0], np.float32) * s1a)
    wkv_h = _pa(np.concatenate([k_w, v_w], axis=1).astype(bf))
    freq = 10000.0 ** (2.0 / H * np.arange(H // 2, dtype=np.float32))
    in_maps = []
    for c in range(8):
        b, r = divmod(c, 4)
        hsel = slice(r * HEADS, (r + 1) * HEADS)
        wq_c = np.asarray(w_q[hsel], np.float32) * s1a[None] * H ** -0.5
        wq_c = _pa(np.concatenate(list(wq_c), axis=1).astype(bf))
        wo_c = _pa(np.concatenate(list(np.asarray(w_o[hsel], np.float32)),
                                  axis=0).astype(bf))
        fsel = slice(r * FS, (r + 1) * FS)
        # wg: [FB, 128, DT, 256] — per F-block, partition-major, gate|up cols
        gate = (np.asarray(w_gating[0][:, fsel], np.float32) * s1f).astype(bf)
        up = (np.asarray(w_gating[1][:, fsel], np.float32) * s1f).astype(bf)
        gate = gate.reshape(DT, 128, FB, 128).transpose(2, 1, 0, 3)
        up = up.reshape(DT, 128, FB, 128).transpose(2, 1, 0, 3)
        wg_c = np.ascontiguousarray(np.concatenate([gate, up], axis=3))
        # wl: [DCH, 128, FB, TCH]
        wl_c = np.asarray(w_linear[fsel], np.float32).astype(bf)
        wl_c = np.ascontiguousarray(
            wl_c.reshape(FB, 128, DCH, TCH).transpose(2, 1, 0, 3))
        pos = np.asarray(positions[b], np.float32)
        rad = pos[None, :] / freq[:, None]                       # [H/2, T]
        xb = np.ascontiguousarray(np.asarray(x[b], np.float32))
        xsl = np.concatenate([xb[ch * TCH + r * BS: ch * TCH + (r + 1) * BS]
                              for ch in range(NCH)], axis=0)
        in_maps.append({
            "x": xb,
            "x_slice": np.ascontiguousarray(xsl),
            "wq": wq_c, "wkv": wkv_h, "wo": wo_c, "wg": wg_c, "wl": wl_c,
            "sin": np.ascontiguousarray(np.sin(rad)),
            "cos": np.ascontiguousarray(np.cos(rad)),
        })
    return in_maps


def assemble(cfg, results, B):
    T, D = cfg["T"], cfg["D"]
    TCH = T // 4
    NCH = T // TCH
    BS = TCH // 4
    out = np.empty((B, T, D), np.float32)
    for c in range(8):
        b, r = divmod(c, 4)
        res = results[c]["out"]
        for ch in range(NCH):
            out[b, ch * TCH + r * BS: ch * TCH + (r + 1) * BS] = \
                res[ch * BS:(ch + 1) * BS]
    return out


# cached compiled program + jitted runner -----------------------------------

_CACHE = {}


def _get_runner(cfg_key, cfg):
    if cfg_key in _CACHE:
        return _CACHE[cfg_key]
    runner = _runner_from_nc(build(cfg))
    _CACHE[cfg_key] = runner
    return runner


def _runner_from_nc(nc):
    import jax
    from jax.experimental.shard_map import shard_map
    from jax.sharding import Mesh, PartitionSpec
    from concourse import bass2jax

    bass2jax.install_neuronx_cc_hook()

    partition_name = (nc.partition_id_tensor.name
                      if nc.partition_id_tensor else None)
    in_names, out_names, out_avals, zero_shapes = [], [], [], []
    for alloc in nc.m.functions[0].allocations:
        if not isinstance(alloc, mybir.MemoryLocationSet):
            continue
        name = alloc.memorylocations[0].name
        if alloc.kind == "ExternalInput":
            if name != partition_name:
                in_names.append(name)
        elif alloc.kind == "ExternalOutput":
            out_names.append(name)
            shape = tuple(alloc.tensor_shape)
            dtype = mybir.dt.np(alloc.dtype)
            out_avals.append(jax.core.ShapedArray(shape, dtype))
            zero_shapes.append((shape, dtype))
    n_params = len(in_names)
    all_in_names = in_names + out_names
    if partition_name is not None:
        all_in_names = all_in_names + [partition_name]

    def _body(*args):
        operands = list(args)
        if partition_name is not None:
            operands.append(bass2jax.partition_id_tensor())
        outs = bass2jax._bass_exec_p.bind(
            *operands,
            out_avals=tuple(out_avals),
            in_names=tuple(all_in_names),
            out_names=tuple(out_names),
            lowering_input_output_aliases=(),
            sim_require_finite=True,
            sim_require_nnan=True,
            nc=nc,
        )
        return tuple(outs)

    n_outs = len(out_names)
    donate = tuple(range(n_params, n_params + n_outs))
    devices = jax.devices()[:8]
    mesh = Mesh(np.asarray(devices), ("core",))
    in_specs = (PartitionSpec("core"),) * (n_params + n_outs)
    out_specs = (PartitionSpec("core"),) * n_outs
    sharded = jax.jit(
        shard_map(_body, mesh=mesh, in_specs=in_specs, out_specs=out_specs,
                  check_rep=False),
        donate_argnums=donate, keep_unused=True)

    class Runner:
        pass

    runner = Runner()
    runner.sharded = sharded
    runner.mesh = mesh
    runner.in_names = in_names
    runner.out_names = out_names
    runner.out_avals = out_avals
    runner.zero_shapes = zero_shapes

    def concat_inputs(in_maps):
        return [np.concatenate([np.asarray(m[name]) for m in in_maps],
                               axis=0) for name in in_names]

    def make_zeros():
        return [np.zeros((8 * s[0], *s[1:]), d) for s, d in zero_shapes]

    def split_outputs(out_arrs):
        return [
            {name: np.asarray(out_arrs[i]).reshape(8, *out_avals[i].shape)[c]
             for i, name in enumerate(out_names)}
            for c in range(8)
        ]

    runner.concat_inputs = concat_inputs
    runner.make_zeros = make_zeros
    runner.split_outputs = split_outputs

    def run(in_maps):
        out_arrs = sharded(*concat_inputs(in_maps), *make_zeros())
        return split_outputs(out_arrs)

    runner.run = run
    return runner


def run_cfg(cfg, inputs):
    cfg_key = tuple(sorted(cfg.items()))
    runner = _get_runner(cfg_key, cfg)
    in_maps = make_in_maps(cfg, **inputs)
    results = runner.run(in_maps)
    return assemble(cfg, results, np.asarray(inputs["x"]).shape[0])


def kernel(**inputs):
    return run_cfg(FULL_CFG, inputs)



# revision 2
# speedup vs baseline: 5.2326x; 5.2326x over previous
"""Trainium2 Bass kernel for dense transformer block nn_Block_68221260529679.

Layout: B=2, T=2048, D=2048, N=8 q-heads, K=1 kv-head, H=256, F=16384.

Sharding (8 NeuronCores): pure sequence parallelism, ZERO collectives.
Core c = 4*b + r handles batch b and the four 128-row tiles
{r, 4+r, 8+r, 12+r}.  Attention for tile 4j+r is computed over a padded
prefix of 512*(j+1) keys, so every core runs an identical instruction
stream (SPMD single-module constraint); the exact causal boundary is
applied via a per-core input bias tile on the last 512-key chunk.  The
padded schedule {512,1024,1536,2048} also balances work across cores.

Every core of a batch group computes rmsnorm(x) and the shared k/v
projection (K=1 kv head) redundantly, then runs q-proj + attention +
o-proj + residual + rmsnorm + full-F MLP + residual for its own 512 rows
entirely locally.  Host scatters the 8 [512, D] slices into the output.

MLP streams the full gate/up (134MB) and down (67MB) weights exactly once
per invocation; down-projection partials accumulate into the residual
x2 rows in SBUF via DVE adds, so PSUM pressure stays bounded while the
weights stream and the final residual add is free.

All matmuls bf16 with fp32 PSUM accumulation; norms/softmax/residuals
fp32.  rmsnorm scales (1+scale) and the q scaling H^-0.5 are folded into
the weights host-side; rope sin/cos tables are precomputed host-side.
"""

from contextlib import ExitStack

import numpy as np
import ml_dtypes

import concourse.bass as bass
import concourse.mybir as mybir
import concourse.tile as tile
from concourse import bacc
from concourse.masks import make_identity

F32 = mybir.dt.float32
BF16 = mybir.dt.bfloat16
AF = mybir.ActivationFunctionType
ALU = mybir.AluOpType
BIG_NEG = -2.3819763e38

FULL_CFG = dict(T=2048, D=2048, H=256, N=8)

FP8 = mybir.dt.float8e4
SCALE_G = 64.0      # gate weights scaled into fp8 range
SCALE_U = 8.0       # up weights scaled into fp8 range (ff carries x8)

T, D, H, N = 2048, 2048, 256, 8
F = 16384
TT, DT = T // 128, D // 128      # 16, 16
FB = F // 128                    # 128 f-blocks
NQ = 4                           # own row tiles per core
ROWS = NQ * 128                  # 512
QCH = 32                         # f-blocks per ffT quarter
NQUART = FB // QCH               # 4
WLFB = 8                         # f-blocks per wl stream chunk
NWLC = QCH // WLFB               # wl chunks per quarter (4)
DSL = 4                          # D column blocks of 512


def build(cfg):
    REPS = cfg.get("reps", 1)
    FP8M = cfg.get("fp8", 0)     # 0: bf16, 1: fp8 gate/up, 2: fp8 all MLP

    nc = bacc.Bacc("TRN2", target_bir_lowering=False, debug=False,
                   num_devices=8)
    x_ext = nc.dram_tensor("x", [T, D], F32, kind="ExternalInput").ap()
    xo_ext = nc.dram_tensor("x_own", [ROWS, D], F32,
                            kind="ExternalInput").ap()
    wq_ext = nc.dram_tensor("wq", [128, DT, N * H], BF16,
                            kind="ExternalInput").ap()
    wkv_ext = nc.dram_tensor("wkv", [128, DT, 2 * H], BF16,
                             kind="ExternalInput").ap()
    wo_ext = nc.dram_tensor("wo", [128, N * H // 128, D], BF16,
                            kind="ExternalInput").ap()
    wg_ext = nc.dram_tensor("wg", [FB, 128, DT, 256],
                            FP8 if FP8M >= 1 else BF16,
                            kind="ExternalInput").ap()
    wl_ext = nc.dram_tensor("wl", [NQUART * NWLC, DSL, 128, WLFB, 512],
                            FP8 if FP8M == 2 else BF16,
                            kind="ExternalInput").ap()
    sinf_ext = nc.dram_tensor("sinf", [H // 2, T], F32,
                              kind="ExternalInput").ap()
    cosf_ext = nc.dram_tensor("cosf", [H // 2, T], F32,
                              kind="ExternalInput").ap()
    sino_ext = nc.dram_tensor("sino", [H // 2, ROWS], F32,
                              kind="ExternalInput").ap()
    coso_ext = nc.dram_tensor("coso", [H // 2, ROWS], F32,
                              kind="ExternalInput").ap()
    mask_ext = nc.dram_tensor("mask", [NQ, 128, 512], F32,
                              kind="ExternalInput").ap()
    out_ext = nc.dram_tensor("out", [ROWS, D], F32,
                             kind="ExternalOutput").ap()

    with tile.TileContext(nc) as tc, ExitStack() as top:
        cons = top.enter_context(tc.tile_pool(name="cons", bufs=1))
        ident = cons.tile([128, 128], BF16)
        make_identity(nc, ident)
        eps = cons.tile([128, 1], F32)
        nc.vector.memset(eps, 1e-6)
        mask_sb = cons.tile([128, NQ, 512], F32)
        for tl in range(NQ):
            nc.sync.dma_start(out=mask_sb[:, tl], in_=mask_ext[tl])

        for _rep in range(REPS):
            with ExitStack() as rep_sc:
                # tiles alive from projections through attention
                attn_acts = rep_sc.enter_context(
                    tc.tile_pool(name="attn_acts", bufs=1))
                kT = attn_acts.tile([128, 2, T], BF16, tag="kT")
                v_sb = attn_acts.tile([128, TT, H], BF16, tag="v")
                qT = attn_acts.tile([128, N, 2, ROWS], BF16, tag="qT")

                def rmsnorm_tile(pool, xt, h):
                    """h = rmsnorm(xt) cast bf16 (scale folded into w)."""
                    ssq = pool.tile([128, 1], F32, tag="ssq")
                    nc.scalar.activation(out=h, in_=xt, func=AF.Square,
                                         accum_out=ssq)
                    rstd = pool.tile([128, 1], F32, tag="rstd")
                    nc.scalar.activation(out=rstd, in_=ssq, func=AF.Sqrt,
                                         bias=eps, scale=1.0 / D)
                    nc.vector.reciprocal(out=rstd, in_=rstd)
                    nc.vector.tensor_scalar_mul(h, xt, rstd)

                def rope_pair(pool, dst, blk1, blk2, x1p, x2p, cs, sn):
                    t1 = pool.tile([128, 512], F32, tag="rp1")
                    t2 = pool.tile([128, 512], F32, tag="rp2")
                    nc.vector.tensor_tensor(t1, x1p, cs, op=ALU.mult)
                    nc.vector.tensor_tensor(t2, x2p, sn, op=ALU.mult)
                    nc.vector.tensor_tensor(dst[:, blk1], t1, t2,
                                            op=ALU.subtract)
                    nc.vector.tensor_tensor(t1, x2p, cs, op=ALU.mult)
                    nc.vector.tensor_tensor(t2, x1p, sn, op=ALU.mult)
                    nc.vector.tensor_tensor(dst[:, blk2], t1, t2,
                                            op=ALU.add)

                # ---- A: rmsnorm(x) + hT, k/v projections + rope(k) ----
                with (
                    tc.tile_pool(name="pa", bufs=2) as pa,
                    tc.tile_pool(name="psa", bufs=2, space="PSUM") as psa,
                ):
                    hT = pa.tile([128, DT, T], BF16, tag="hT", bufs=1)
                    wkvs = pa.tile([128, DT, 2 * H], BF16, tag="wkvs",
                                   bufs=1)
                    nc.sync.dma_start(out=wkvs, in_=wkv_ext)
                    sinf = pa.tile([128, T], F32, tag="sinf", bufs=1)
                    nc.sync.dma_start(out=sinf, in_=sinf_ext)
                    cosf = pa.tile([128, T], F32, tag="cosf", bufs=1)
                    nc.sync.dma_start(out=cosf, in_=cosf_ext)

                    for ch in range(4):
                        for t in range(4 * ch, 4 * ch + 4):
                            tsl = slice(t * 128, (t + 1) * 128)
                            xt = pa.tile([128, D], F32, tag="xt")
                            nc.sync.dma_start(out=xt, in_=x_ext[tsl])
                            h = pa.tile([128, D], BF16, tag="h")
                            rmsnorm_tile(pa, xt, h)
                            for kd in range(DT):
                                pt = psa.tile([128, 128], BF16, tag="tp")
                                nc.tensor.transpose(
                                    pt, h[:, kd * 128:(kd + 1) * 128], ident)
                                nc.vector.tensor_copy(hT[:, kd, tsl], pt)
                        csl = slice(ch * 512, (ch + 1) * 512)
                        p1 = psa.tile([128, 512], F32, tag="qk1")
                        p2 = psa.tile([128, 512], F32, tag="qk2")
                        for kd in range(DT):
                            nc.tensor.matmul(p1, wkvs[:, kd, 0:128],
                                             hT[:, kd, csl],
                                             start=kd == 0, stop=kd == DT - 1)
                        for kd in range(DT):
                            nc.tensor.matmul(p2, wkvs[:, kd, 128:256],
                                             hT[:, kd, csl],
                                             start=kd == 0, stop=kd == DT - 1)
                        rope_pair(pa, kT[:, :, csl], 0, 1, p1, p2,
                                  cosf[:, csl], sinf[:, csl])
                        for st in range(4 * ch, 4 * ch + 4):
                            pv = psa.tile([128, H], F32, tag="vv")
                            for kd in range(DT):
                                nc.tensor.matmul(
                                    pv, hT[:, kd, st * 128:(st + 1) * 128],
                                    wkvs[:, kd, H:2 * H],
                                    start=kd == 0, stop=kd == DT - 1)
                            nc.vector.tensor_copy(v_sb[:, st], pv)

                # ---- B: own-row norm + q-proj + rope(q) ----
                with (
                    tc.tile_pool(name="pb", bufs=2) as pb_,
                    tc.tile_pool(name="psb", bufs=2, space="PSUM") as psb,
                ):
                    sino = pb_.tile([128, ROWS], F32, tag="sino", bufs=1)
                    nc.sync.dma_start(out=sino, in_=sino_ext)
                    coso = pb_.tile([128, ROWS], F32, tag="coso", bufs=1)
                    nc.sync.dma_start(out=coso, in_=coso_ext)
                    hTo = pb_.tile([128, DT, ROWS], BF16, tag="hTo", bufs=1)
                    for tl in range(NQ):
                        rsl = slice(tl * 128, (tl + 1) * 128)
                        xo = pb_.tile([128, D], F32, tag="xo")
                        nc.sync.dma_start(out=xo, in_=xo_ext[rsl])
                        ho = pb_.tile([128, D], BF16, tag="ho")
                        rmsnorm_tile(pb_, xo, ho)
                        for kd in range(DT):
                            pt = psb.tile([128, 128], BF16, tag="tp")
                            nc.tensor.transpose(
                                pt, ho[:, kd * 128:(kd + 1) * 128], ident)
                            nc.vector.tensor_copy(hTo[:, kd, rsl], pt)
                    wqs = pb_.tile([128, DT, N * H], BF16, tag="wqs", bufs=1)
                    nc.sync.dma_start(out=wqs, in_=wq_ext)
                    for hd in range(N):
                        p1 = psb.tile([128, 512], F32, tag="qk1")
                        p2 = psb.tile([128, 512], F32, tag="qk2")
                        c0 = (2 * hd) * 128
                        c1 = (2 * hd + 1) * 128
                        for kd in range(DT):
                            nc.tensor.matmul(p1, wqs[:, kd, c0:c0 + 128],
                                             hTo[:, kd, :],
                                             start=kd == 0, stop=kd == DT - 1)
                        for kd in range(DT):
                            nc.tensor.matmul(p2, wqs[:, kd, c1:c1 + 128],
                                             hTo[:, kd, :],
                                             start=kd == 0, stop=kd == DT - 1)
                        rope_pair(pb_, qT[:, hd], 0, 1, p1, p2, coso, sino)

                # tiles alive from attention through the MLP
                acts = rep_sc.enter_context(
                    tc.tile_pool(name="acts", bufs=1))
                x2t = acts.tile([128, NQ, D], F32, tag="x2t")
                h2To = acts.tile([128, DT, ROWS], BF16, tag="h2To")

                # ---- P3: attention + o-proj + residual + ffn-norm ----
                with (
                    tc.tile_pool(name="p3", bufs=2) as p3,
                    tc.tile_pool(name="ps3", bufs=2, space="PSUM") as ps3,
                ):
                    wos = p3.tile([128, N * H // 128, D], BF16, tag="wos",
                                  bufs=1)
                    nc.sync.dma_start(out=wos, in_=wo_ext)

                    for tl in range(NQ):
                        SP = 512 * (tl + 1)
                        nS = SP // 128
                        rsl = slice(tl * 128, (tl + 1) * 128)
                        encT = p3.tile([128, N * 2, 128], BF16, tag="encT",
                                       bufs=2)
                        pend = []

                        def flush(hd, pb, nS=nS, encT=encT):
                            pT = p3.tile([128, 16, 128], BF16, tag="pT")
                            for s in range(nS):
                                tp = ps3.tile([128, 128], BF16, tag="tp")
                                nc.tensor.transpose(
                                    tp, pb[:, s * 128:(s + 1) * 128], ident)
                                nc.vector.tensor_copy(pT[:, s], tp)
                            for m in range(2):
                                pe_ = ps3.tile([128, 128], F32, tag="av")
                                for s in range(nS):
                                    nc.tensor.matmul(
                                        pe_,
                                        v_sb[:, s, m * 128:(m + 1) * 128],
                                        pT[:, s],
                                        start=s == 0, stop=s == nS - 1)
                                nc.vector.tensor_copy(encT[:, 2 * hd + m],
                                                      pe_)

                        for hd in range(N):
                            lg = p3.tile([128, 2048], F32, tag="lg", bufs=1)
                            for sc in range(tl + 1):
                                ssl = slice(sc * 512, (sc + 1) * 512)
                                pl = ps3.tile([128, 512], F32, tag="lgp")
                                nc.tensor.matmul(pl, qT[:, hd, 0, rsl],
                                                 kT[:, 0, ssl],
                                                 start=True, stop=False)
                                nc.tensor.matmul(pl, qT[:, hd, 1, rsl],
                                                 kT[:, 1, ssl],
                                                 start=False, stop=True)
                                if sc < tl:
                                    nc.vector.tensor_copy(lg[:, ssl], pl)
                                else:
                                    nc.vector.tensor_tensor(
                                        lg[:, ssl], pl, mask_sb[:, tl],
                                        op=ALU.add)
                            nmax = p3.tile([128, 1], F32, tag="nmax")
                            nc.vector.tensor_reduce(
                                nmax, lg[:, :SP], axis=mybir.AxisListType.X,
                                op=ALU.max, negate=True)
                            sume = p3.tile([128, 1], F32, tag="sume")
                            nc.scalar.activation(
                                out=lg[:, :SP], in_=lg[:, :SP], func=AF.Exp,
                                bias=nmax, accum_out=sume)
                            rsum = p3.tile([128, 1], F32, tag="rsum")
                            nc.vector.reciprocal(rsum, sume)
                            pb = p3.tile([128, 2048], BF16, tag="pb")
                            nc.vector.tensor_scalar_mul(
                                pb[:, :SP], lg[:, :SP], rsum)
                            pend.append((hd, pb))
                            if len(pend) == 2:
                                flush(*pend.pop(0))
                        flush(*pend.pop(0))

                        # o-proj + residual (DVE adds psum onto x rows)
                        xo = p3.tile([128, D], F32, tag="xo3", bufs=1)
                        nc.sync.dma_start(out=xo, in_=xo_ext[rsl])
                        for d in range(DSL):
                            dsl = slice(d * 512, (d + 1) * 512)
                            po = ps3.tile([128, 512], F32, tag="oproj")
                            for nh in range(N * 2):
                                nc.tensor.matmul(
                                    po, encT[:, nh], wos[:, nh, dsl],
                                    start=nh == 0, stop=nh == N * 2 - 1)
                            nc.vector.tensor_tensor(
                                x2t[:, tl, dsl], xo[:, dsl], po, op=ALU.add)
                        h2 = p3.tile([128, D], BF16, tag="h2", bufs=1)
                        rmsnorm_tile(p3, x2t[:, tl], h2)
                        for kd in range(DT):
                            pt = ps3.tile([128, 128], BF16, tag="tp")
                            nc.tensor.transpose(
                                pt, h2[:, kd * 128:(kd + 1) * 128], ident)
                            nc.vector.tensor_copy(h2To[:, kd, rsl], pt)

                # ---- P4: MLP over full F, weights streamed once ----
                with (
                    tc.tile_pool(name="p4", bufs=2) as p4,
                    tc.tile_pool(name="ps4", bufs=2, space="PSUM") as ps4,
                ):
                    ff_dt = FP8 if FP8M == 2 else BF16
                    wg_dt = FP8 if FP8M >= 1 else BF16
                    wl_dt = FP8 if FP8M == 2 else BF16
                    DR = mybir.MatmulPerfMode.DoubleRow
                    ffT = p4.tile([128, QCH, ROWS], ff_dt, tag="ffT", bufs=1)
                    if FP8M >= 1:
                        h2To8 = p4.tile([128, DT, ROWS], FP8, tag="h2To8",
                                        bufs=1)
                        nc.vector.tensor_copy(h2To8, h2To)
                        h2m = h2To8
                    else:
                        h2m = h2To
                    for q in range(NQUART):
                        for fj in range(QCH):
                            fb = q * QCH + fj
                            wgf = p4.tile([128, DT, 256], wg_dt, tag="wgf",
                                          bufs=3)
                            nc.sync.dma_start(out=wgf, in_=wg_ext[fb])
                            gps = ps4.tile([128, 512], F32, tag="gps")
                            ups = ps4.tile([128, 512], F32, tag="ups")
                            if FP8M >= 1:
                                for kp in range(DT // 2):
                                    kk = slice(2 * kp, 2 * kp + 2)
                                    nc.tensor.matmul(
                                        gps, wgf[:, kk, 0:128],
                                        h2m[:, kk, :], perf_mode=DR,
                                        start=kp == 0,
                                        stop=kp == DT // 2 - 1)
                                for kp in range(DT // 2):
                                    kk = slice(2 * kp, 2 * kp + 2)
                                    nc.tensor.matmul(
                                        ups, wgf[:, kk, 128:256],
                                        h2m[:, kk, :], perf_mode=DR,
                                        start=kp == 0,
                                        stop=kp == DT // 2 - 1)
                            else:
                                for kd in range(DT):
                                    nc.tensor.matmul(gps, wgf[:, kd, 0:128],
                                                     h2m[:, kd, :],
                                                     start=kd == 0,
                                                     stop=kd == DT - 1)
                                for kd in range(DT):
                                    nc.tensor.matmul(ups,
                                                     wgf[:, kd, 128:256],
                                                     h2m[:, kd, :],
                                                     start=kd == 0,
                                                     stop=kd == DT - 1)
                            ga = p4.tile([128, 512], F32, tag="ga")
                            nc.scalar.activation(
                                out=ga, in_=gps, func=AF.Gelu_apprx_tanh,
                                scale=(1.0 / SCALE_G) if FP8M >= 1 else 1.0)
                            nc.vector.tensor_tensor(ffT[:, fj], ga, ups,
                                                    op=ALU.mult)
                        # fold this quarter's down-proj into x2t
                        for d in range(DSL):
                            wlcs = []
                            for c in range(NWLC):
                                wlc = p4.tile([128, WLFB, 512], wl_dt,
                                              tag="wlc", bufs=6)
                                nc.sync.dma_start(
                                    out=wlc, in_=wl_ext[q * NWLC + c, d])
                                wlcs.append(wlc)
                            for tl in range(NQ):
                                rsl = slice(tl * 128, (tl + 1) * 128)
                                dp = ps4.tile([128, 512], F32, tag="dp")
                                if FP8M == 2:
                                    for c in range(NWLC):
                                        for fp in range(WLFB // 2):
                                            ff2 = slice(
                                                c * WLFB + 2 * fp,
                                                c * WLFB + 2 * fp + 2)
                                            w2 = slice(2 * fp, 2 * fp + 2)
                                            nc.tensor.matmul(
                                                dp, ffT[:, ff2, rsl],
                                                wlcs[c][:, w2], perf_mode=DR,
                                                start=(c == 0 and fp == 0),
                                                stop=(c == NWLC - 1
                                                      and fp == WLFB // 2 - 1))
                                else:
                                    for c in range(NWLC):
                                        for fj in range(WLFB):
                                            nc.tensor.matmul(
                                                dp,
                                                ffT[:, c * WLFB + fj, rsl],
                                                wlcs[c][:, fj],
                                                start=(c == 0 and fj == 0),
                                                stop=(c == NWLC - 1
                                                      and fj == WLFB - 1))
                                dst = x2t[:, tl, d * 512:(d + 1) * 512]
                                if FP8M == 2:
                                    dpf = p4.tile([128, 512], F32, tag="dpf")
                                    nc.scalar.activation(
                                        out=dpf, in_=dp, func=AF.Copy,
                                        scale=1.0 / (SCALE_G * SCALE_U))
                                    nc.vector.tensor_tensor(dst, dst, dpf,
                                                            op=ALU.add)
                                else:
                                    nc.vector.tensor_tensor(dst, dst, dp,
                                                            op=ALU.add)
                    for tl in range(NQ):
                        nc.sync.dma_start(
                            out=out_ext[tl * 128:(tl + 1) * 128],
                            in_=x2t[:, tl])
    nc.compile()
    return nc


# ---------------------------------------------------------------------------
# host side
# ---------------------------------------------------------------------------

def _pa(w, inner=128):
    """[A*128, M] -> [128, A, M] partition-major layout."""
    a = w.shape[0] // inner
    return np.ascontiguousarray(
        w.reshape(a, inner, w.shape[1]).transpose(1, 0, 2))


def make_in_maps(cfg, x, positions, attn_mask, scale_attn, w_q, w_kv, w_o,
                 scale_ffn, w_gating, w_linear):
    bf = ml_dtypes.bfloat16
    s1a = (1.0 + np.asarray(scale_attn, np.float32))[:, None]
    s1f = (1.0 + np.asarray(scale_ffn, np.float32))[:, None]
    k_w = np.asarray(w_kv[0, 0], np.float32) * s1a
    v_w = np.asarray(w_kv[1, 0], np.float32) * s1a
    wkv_h = _pa(np.concatenate([k_w, v_w], axis=1).astype(bf))
    wq_all = np.asarray(w_q, np.float32) * s1a[None] * H ** -0.5
    wq_h = _pa(np.concatenate(list(wq_all), axis=1).astype(bf))
    wo_h = _pa(np.concatenate(list(np.asarray(w_o, np.float32)),
                              axis=0).astype(bf))
    fp8m = cfg.get("fp8", 0)
    f8 = ml_dtypes.float8_e4m3
    gate = np.asarray(w_gating[0], np.float32) * s1f
    up = np.asarray(w_gating[1], np.float32) * s1f
    wl_f = np.asarray(w_linear, np.float32)
    if fp8m >= 1:
        gate = (gate * SCALE_G).astype(f8)
        up = (up * SCALE_U).astype(f8)
        wl_f = wl_f * SCALE_G if fp8m == 2 else wl_f / SCALE_U
    else:
        gate = gate.astype(bf)
        up = up.astype(bf)
    # [D, F] -> [FB, 128, DT, 128] each, concat -> [FB, 128, DT, 256]
    gate = gate.reshape(DT, 128, FB, 128).transpose(2, 1, 0, 3)
    up = up.reshape(DT, 128, FB, 128).transpose(2, 1, 0, 3)
    wg_h = np.ascontiguousarray(np.concatenate([gate, up], axis=3))
    # wl [F, D] -> [NQUART*NWLC, DSL, 128, WLFB, 512]
    wl_h = wl_f.astype(f8 if fp8m == 2 else bf)
    wl_h = np.ascontiguousarray(
        wl_h.reshape(NQUART * NWLC, WLFB, 128, DSL, 512)
        .transpose(0, 3, 2, 1, 4))
    freq = 10000.0 ** (2.0 / H * np.arange(H // 2, dtype=np.float32))

    in_maps = []
    for c in range(8):
        b, r = divmod(c, 4)
        tiles = [r, 4 + r, 8 + r, 12 + r]
        xb = np.ascontiguousarray(np.asarray(x[b], np.float32))
        x_own = np.concatenate([xb[ti * 128:(ti + 1) * 128] for ti in tiles],
                               axis=0)
        pos = np.asarray(positions[b], np.float32)
        radf = pos[None, :] / freq[:, None]                      # [H/2, T]
        pos_own = np.concatenate(
            [pos[ti * 128:(ti + 1) * 128] for ti in tiles])
        rado = pos_own[None, :] / freq[:, None]                  # [H/2, 512]
        mask = np.zeros((NQ, 128, 512), np.float32)
        for j, ti in enumerate(tiles):
            cs = 512 * j                      # global col of chunk start
            cols = cs + np.arange(512)[None, :]
            rows = ti * 128 + np.arange(128)[:, None]
            mask[j] = np.where(cols <= rows, 0.0, BIG_NEG)
        in_maps.append({
            "x": xb,
            "x_own": np.ascontiguousarray(x_own),
            "wq": wq_h, "wkv": wkv_h, "wo": wo_h, "wg": wg_h, "wl": wl_h,
            "sinf": np.ascontiguousarray(np.sin(radf)),
            "cosf": np.ascontiguousarray(np.cos(radf)),
            "sino": np.ascontiguousarray(np.sin(rado)),
            "coso": np.ascontiguousarray(np.cos(rado)),
            "mask": mask,
        })
    return in_maps


def assemble(cfg, results, B):
    out = np.empty((B, T, D), np.float32)
    for c in range(8):
        b, r = divmod(c, 4)
        tiles = [r, 4 + r, 8 + r, 12 + r]
        res = results[c]["out"]
        for j, ti in enumerate(tiles):
            out[b, ti * 128:(ti + 1) * 128] = res[j * 128:(j + 1) * 128]
    return out


_CACHE = {}


def _get_runner(cfg_key, cfg):
    if cfg_key in _CACHE:
        return _CACHE[cfg_key]
    runner = _runner_from_nc(build(cfg))
    _CACHE[cfg_key] = runner
    return runner


def _runner_from_nc(nc):
    import jax
    from jax.experimental.shard_map import shard_map
    from jax.sharding import Mesh, PartitionSpec
    from concourse import bass2jax

    bass2jax.install_neuronx_cc_hook()

    partition_name = (nc.partition_id_tensor.name
                      if nc.partition_id_tensor else None)
    in_names, out_names, out_avals, zero_shapes = [], [], [], []
    for alloc in nc.m.functions[0].allocations:
        if not isinstance(alloc, mybir.MemoryLocationSet):
            continue
        name = alloc.memorylocations[0].name
        if alloc.kind == "ExternalInput":
            if name != partition_name:
                in_names.append(name)
        elif alloc.kind == "ExternalOutput":
            out_names.append(name)
            shape = tuple(alloc.tensor_shape)
            dtype = mybir.dt.np(alloc.dtype)
            out_avals.append(jax.core.ShapedArray(shape, dtype))
            zero_shapes.append((shape, dtype))
    n_params = len(in_names)
    all_in_names = in_names + out_names
    if partition_name is not None:
        all_in_names = all_in_names + [partition_name]

    def _body(*args):
        operands = list(args)
        if partition_name is not None:
            operands.append(bass2jax.partition_id_tensor())
        outs = bass2jax._bass_exec_p.bind(
            *operands,
            out_avals=tuple(out_avals),
            in_names=tuple(all_in_names),
            out_names=tuple(out_names),
            lowering_input_output_aliases=(),
            sim_require_finite=True,
            sim_require_nnan=True,
            nc=nc,
        )
        return tuple(outs)

    n_outs = len(out_names)
    donate = tuple(range(n_params, n_params + n_outs))
    devices = jax.devices()[:8]
    mesh = Mesh(np.asarray(devices), ("core",))
    in_specs = (PartitionSpec("core"),) * (n_params + n_outs)
    out_specs = (PartitionSpec("core"),) * n_outs
    sharded = jax.jit(
        shard_map(_body, mesh=mesh, in_specs=in_specs, out_specs=out_specs,
                  check_rep=False),
        donate_argnums=donate, keep_unused=True)

    class Runner:
        pass

    runner = Runner()
    runner.sharded = sharded
    runner.mesh = mesh
    runner.in_names = in_names
    runner.out_names = out_names
    runner.out_avals = out_avals
    runner.zero_shapes = zero_shapes

    def concat_inputs(in_maps):
        return [np.concatenate([np.asarray(m[name]) for m in in_maps],
                               axis=0) for name in in_names]

    def make_zeros():
        return [np.zeros((8 * s[0], *s[1:]), d) for s, d in zero_shapes]

    def split_outputs(out_arrs):
        return [
            {name: np.asarray(out_arrs[i]).reshape(8, *out_avals[i].shape)[c]
             for i, name in enumerate(out_names)}
            for c in range(8)
        ]

    runner.concat_inputs = concat_inputs
    runner.make_zeros = make_zeros
    runner.split_outputs = split_outputs

    def run(in_maps):
        out_arrs = sharded(*concat_inputs(in_maps), *make_zeros())
        return split_outputs(out_arrs)

    runner.run = run
    return runner




def run_cfg(cfg, inputs):
    cfg_key = tuple(sorted(cfg.items()))
    runner = _get_runner(cfg_key, cfg)
    in_maps = make_in_maps(cfg, **inputs)
    results = runner.run(in_maps)
    return assemble(cfg, results, np.asarray(inputs["x"]).shape[0])


def kernel(**inputs):
    return run_cfg(FULL_CFG, inputs)
